# revision 1
# baseline (speedup 1.0000x reference)
"""TRN2 Bass kernel for nn_CNN_transformer_hr_xyz_41051297415299.

Reference model (B=32, C=512, D=512, H=8, DFF=2048, K=7), per batch element:
    query_in = causal_conv_in(x)                 # conv over last axis t, mixing C
    xn       = LN0(query_in)                     # over t, (x-m)/(std+eps), ddof=1
    q = conv_q(query_in); k = conv_k(xn); v = conv_v(xn)
    heads split the t axis (8 x 64); attention over the C axis
    o  = softmax(q k^T / 8) v   -> (C, D)
    y  = conv_o(o);  h1 = 2y
    hn = LN1(h1)  ==  LN(y) with eps/2
    out = 2 * (relu(hn @ w1 + b1) @ w2 + b2)

Sharding: data-parallel over batch, 4 per NeuronCore, no collectives.
All matmuls run as float32r (TF32-like, full PE rate at N>=256, ~1.5e-4 rel err).

Device layout notes (per batch element b):
    std layout  = [channel c (partitions, 4 chunks), t (free)]
    T   layout  = [t (partitions, 4 chunks), channel (free)]
    x, query_in, xn, o_full : std, padded free dim 6+512 (causal left pad)
    qT, kT  : T (conv emitted transposed: lhsT=activation window, rhs=weight)
    v_aug   : [c (part), chunk, head, 65]  (64 v cols + ones col -> softmax
              denominator accumulates in the same matmul as o = p @ v)
    ff1T    : [dff (16 chunks), c]; ff2 emits std [c, d].
"""
import numpy as np
from contextlib import ExitStack

try:
    import concourse.bass as bass
except ImportError:  # pragma: no cover - path fallback for bare containers
    import sys
    for _p in ("/opt/trn_rl_repo", "/root/.axon_site/_ro/trn_rl_repo"):
        if _p not in sys.path:
            sys.path.insert(0, _p)
    import concourse.bass as bass

import concourse.mybir as mybir
import concourse.tile as tile
from concourse import bacc
from concourse.bass_utils import run_bass_kernel_spmd
from concourse.masks import make_identity

B, C, D, H, DFF, KW = 32, 512, 512, 8, 2048, 7
NCORES = 8
BL = B // NCORES          # 4 batch elements per core
DH = D // H               # 64
PAD = KW - 1              # 6
EPS = 1e-6
F32 = mybir.dt.float32
F32R = mybir.dt.float32r
import os as _os
QK_MODE = _os.environ.get("K_QK", "T")        # "T" (direct transposed conv) | "stdT"
O_MODE = _os.environ.get("K_O", "std66")      # "std66" | "oT"
WBUFS = int(_os.environ.get("K_WBUFS", "5"))
AF = mybir.ActivationFunctionType
ALU = mybir.AluOpType


def _conv_w_host(w):
    """(cout, cin, KW) -> (4, 128, KW*512): [ci][p][k*512+cout]."""
    return np.ascontiguousarray(
        w.transpose(1, 2, 0).reshape(4, 128, KW * C).astype(np.float32))


def build_nc(reps=1):
    nc = bacc.Bacc("TRN2", target_bir_lowering=False, debug=False)

    xp = nc.declare_dram_parameter("xp", [BL, 4, 128, PAD + D], F32R, isOutput=False)
    wps = {n: nc.declare_dram_parameter(n, [4, 128, KW * C], F32R, isOutput=False)
           for n in ("win", "wq", "wk", "wv", "wo")}
    w1p = nc.declare_dram_parameter("w1p", [4, 128, DFF], F32R, isOutput=False)
    w2p = nc.declare_dram_parameter("w2p", [16, 128, D], F32R, isOutput=False)
    browp = nc.declare_dram_parameter("browp", [1, 3 * 512], F32R, isOutput=False)
    onecp = nc.declare_dram_parameter("onecp", [1, 128], F32R, isOutput=False)
    bppp = nc.declare_dram_parameter("bppp", [128, 36], F32, isOutput=False)
    lnp = {n: nc.declare_dram_parameter(n, [128, D], F32, isOutput=False)
           for n in ("ln0g", "ln0b", "ln1g", "ln1b")}
    onesp = nc.declare_dram_parameter("onesp", [128, 4, 8, 2], F32, isOutput=False)
    zerosp = nc.declare_dram_parameter("zerosp", [128, 4, PAD], F32, isOutput=False)
    outp = nc.declare_dram_parameter("outp", [BL, C, D], F32, isOutput=True)
    hnTd = nc.dram_tensor("hnTd", [BL, 4, 128, D], F32R)

    with tile.TileContext(nc) as tc, ExitStack() as octx:
        cp = octx.enter_context(tc.tile_pool(name="consts", bufs=1))
        pmm = octx.enter_context(tc.tile_pool(name="pmm", bufs=4, space="PSUM"))

        def ctile(name, shape, dtype, src):
            t = cp.tile(shape, dtype, tag=name, name=name)
            nc.sync.dma_start(t[:], src)
            return t

        brow = ctile("brow", [1, 3 * 512], F32R, browp.ap())
        onec = ctile("onec", [1, 128], F32R, onecp.ap())
        bpp = ctile("bpp", [128, 36], F32, bppp.ap())
        ln_t = {n: ctile(n, [128, D], F32, lnp[n].ap()) for n in lnp}
        ones_t = ctile("ones", [128, 4, 8, 2], F32, onesp.ap())
        zeros_t = ctile("zeros", [128, 4, PAD], F32, zerosp.ap())
        ident = cp.tile([128, 128], F32, tag="ident", name="ident")
        make_identity(nc, ident[:])

        def load_w(pool, param, label):
            ts = []
            for ci in range(4):
                t = pool.tile([128, KW * C], F32R, tag="w", name=f"{label}{ci}")
                nc.sync.dma_start(t[:], param.ap()[ci])
                ts.append(t)
            return ts

        def conv_std(bs, wt, src, writer):
            """std conv: out[cout, t] accumulated over (cin chunk, tap);
            weight lhsT reused across the batch pair."""
            for oc in range(4):
                ps = {b: pmm.tile([128, D], F32, tag="mm", name=f"cs{oc}{b}")
                      for b in bs}
                for ci in range(4):
                    for k in range(KW):
                        lhsT = wt[ci][:, k * C + oc * 128: k * C + oc * 128 + 128]
                        for b in bs:
                            nc.tensor.matmul(
                                ps[b][:], lhsT, src[b][:, ci, k:k + D],
                                start=(ci == 0 and k == 0),
                                stop=(ci == 3 and k == KW - 1))
                for b in bs:
                    writer(b, oc, ps[b])

        def conv_T(bs, wt, src, brow_off, dst):
            """transposed conv: out[t, cout]; rank-1 bias matmul first."""
            for tcn in range(4):
                ps = {b: pmm.tile([128, D], F32, tag="mm", name=f"cT{tcn}{b}")
                      for b in bs}
                for b in bs:
                    nc.tensor.matmul(ps[b][:], onec[:],
                                     brow[:, brow_off:brow_off + D],
                                     start=True, stop=False)
                for ci in range(4):
                    for k in range(KW):
                        rhs = wt[ci][:, k * C:(k + 1) * C]
                        for b in bs:
                            lhsT = src[b][:, ci, tcn * 128 + k: tcn * 128 + k + 128]
                            nc.tensor.matmul(ps[b][:], lhsT, rhs, start=False,
                                             stop=(ci == 3 and k == KW - 1))
                for b in bs:
                    nc.vector.tensor_copy(dst[b][:, tcn, :], ps[b][:])

        def transpose_512(src_t, dst_t, label):
            """[c-chunks, t] std tile -> [t-chunks, c] tile via 16 PE transposes."""
            for tcn in range(4):
                for cc in range(4):
                    tp = patt.tile([128, 128], F32, tag="att",
                                   name=f"tp{label}{tcn}{cc}")
                    nc.tensor.transpose(
                        tp[:], src_t[:, cc, tcn * 128:(tcn + 1) * 128], ident[:])
                    nc.vector.tensor_copy(
                        dst_t[:, tcn, cc * 128:(cc + 1) * 128], tp[:])

        def emit_ln(bs, lnw, stat, src, dst, g_t, b_t, eps, padded_src):
            for b in bs:
                for c in range(4):
                    sv = (src[b][:, c, PAD:PAD + D] if padded_src
                          else src[b][:, c, :])
                    sv = sv.bitcast(F32) if sv.dtype == F32R else sv
                    sm = stat.tile([128, 1], F32, tag="st", name=f"sm{b}{c}")
                    nc.vector.reduce_sum(sm[:], sv, axis=mybir.AxisListType.X)
                    mn = stat.tile([128, 1], F32, tag="st", name=f"mn{b}{c}")
                    nc.scalar.mul(mn[:], sm[:], 1.0 / D)
                    cent = lnw.tile([128, D], F32, tag="lw", name=f"ce{b}{c}")
                    nc.vector.tensor_scalar(cent[:], sv, mn[:], None,
                                            op0=ALU.subtract)
                    scr = lnw.tile([128, D], F32, tag="lw", name=f"sc{b}{c}")
                    sq = stat.tile([128, 1], F32, tag="st", name=f"sq{b}{c}")
                    nc.scalar.activation(scr[:], cent[:], AF.Square,
                                         accum_out=sq[:])
                    st = stat.tile([128, 1], F32, tag="st", name=f"sd{b}{c}")
                    nc.scalar.activation(st[:], sq[:], AF.Sqrt,
                                         scale=1.0 / (D - 1))
                    dn = stat.tile([128, 1], F32, tag="st", name=f"dn{b}{c}")
                    nc.vector.tensor_scalar_add(dn[:], st[:], eps)
                    iv = stat.tile([128, 1], F32, tag="st", name=f"iv{b}{c}")
                    nc.vector.reciprocal(iv[:], dn[:])
                    tmp = lnw.tile([128, D], F32, tag="lw", name=f"tm{b}{c}")
                    nc.vector.scalar_tensor_tensor(
                        tmp[:], in0=cent[:], scalar=iv[:], in1=g_t[:],
                        op0=ALU.mult, op1=ALU.mult)
                    dv = (dst[b][:, c, PAD:PAD + D] if padded_src
                          else dst[b][:, c, :])
                    nc.vector.tensor_add(dv, tmp[:], b_t[:])

        def zero_pads(t):
            nc.scalar.copy(t[:, :, 0:PAD], zeros_t[:])

        for _rep in range(reps):
            # ======== two passes over batch pairs ========
            with ExitStack() as pctx:
                wconv = pctx.enter_context(tc.tile_pool(name="wconv", bufs=WBUFS))
                act = pctx.enter_context(tc.tile_pool(name="act", bufs=8))
                qsp = pctx.enter_context(tc.tile_pool(name="qsp", bufs=2))
                expp = pctx.enter_context(tc.tile_pool(name="expp", bufs=3))
                lnw = pctx.enter_context(tc.tile_pool(name="lnw", bufs=2))
                stat = pctx.enter_context(tc.tile_pool(name="stat", bufs=16))
                hpool = pctx.enter_context(tc.tile_pool(name="hpool", bufs=2))
                patt = pctx.enter_context(
                    tc.tile_pool(name="patt", bufs=4, space="PSUM"))

                for pi in range(BL // 2):
                    bs = [2 * pi, 2 * pi + 1]
                    # s1: conv_in
                    x_t = {}
                    for b in bs:
                        x_t[b] = act.tile([128, 4, PAD + D], F32R, tag="a",
                                          name=f"x{b}")
                        nc.sync.dma_start(
                            x_t[b][:], xp.ap()[b].rearrange("c p t -> p c t"))
                    w_t = load_w(wconv, wps["win"], f"win{pi}")
                    qin = {}
                    for b in bs:
                        qin[b] = act.tile([128, 4, PAD + D], F32R, tag="a",
                                          name=f"qin{b}")
                        zero_pads(qin[b])

                    def wr_qin(b, oc, ps):
                        nc.scalar.activation(qin[b][:, oc, PAD:PAD + D], ps[:],
                                             AF.Identity, bias=bpp[:, oc:oc + 1])
                    conv_std(bs, w_t, x_t, wr_qin)

                    # s2: LN0
                    xn = {}
                    for b in bs:
                        xn[b] = act.tile([128, 4, PAD + D], F32R, tag="a",
                                         name=f"xn{b}")
                        zero_pads(xn[b])
                    emit_ln(bs, lnw, stat, qin, xn, ln_t["ln0g"], ln_t["ln0b"],
                            EPS, padded_src=True)

                    # s3/s4: conv_q / conv_k -> qT, kT
                    qT = {b: act.tile([128, 4, D], F32R, tag="a",
                                      name=f"qT{b}") for b in bs}
                    kT = {b: act.tile([128, 4, D], F32R, tag="a",
                                      name=f"kT{b}") for b in bs}
                    if QK_MODE == "T":
                        w_t = load_w(wconv, wps["wq"], f"wq{pi}")
                        conv_T(bs, w_t, qin, 0, qT)
                        w_t = load_w(wconv, wps["wk"], f"wk{pi}")
                        conv_T(bs, w_t, xn, 512, kT)
                    else:
                        w_t = load_w(wconv, wps["wq"], f"wq{pi}")
                        qstd = {b: qsp.tile([128, 4, D], F32, tag="qs",
                                            name=f"qstd{b}") for b in bs}

                        def wr_q(b, oc, ps):
                            nc.scalar.activation(qstd[b][:, oc, :], ps[:],
                                                 AF.Identity,
                                                 bias=bpp[:, 28 + oc:29 + oc])
                        conv_std(bs, w_t, qin, wr_q)
                        for b in bs:
                            transpose_512(qstd[b], qT[b], f"q{b}")
                        w_t = load_w(wconv, wps["wk"], f"wk{pi}")
                        kstd = {b: qsp.tile([128, 4, D], F32, tag="qs",
                                            name=f"kstd{b}") for b in bs}

                        def wr_k(b, oc, ps):
                            nc.scalar.activation(kstd[b][:, oc, :], ps[:],
                                                 AF.Identity,
                                                 bias=bpp[:, 32 + oc:33 + oc])
                        conv_std(bs, w_t, xn, wr_k)
                        for b in bs:
                            transpose_512(kstd[b], kT[b], f"k{b}")

                    # s5: conv_v -> v_aug; per-head cols: [ones, v0..63, zero]
                    w_t = load_w(wconv, wps["wv"], f"wv{pi}")
                    vaug = {}
                    for b in bs:
                        vaug[b] = act.tile([128, 4, H, DH + 2], F32R, tag="a",
                                           name=f"vaug{b}")
                        nc.scalar.copy(vaug[b][:, :, :, DH:DH + 2],
                                       ones_t[:])

                    def wr_v(b, oc, ps):
                        nc.scalar.activation(
                            vaug[b][:, oc, :, 0:DH],
                            ps[:].rearrange("p (h dd) -> p h dd", h=H),
                            AF.Identity, bias=bpp[:, 4 + oc:5 + oc])
                    conv_std(bs, w_t, xn, wr_v)

                    # s6: attention; o computed transposed (v_aug as lhsT),
                    # colsum rides along as out row 0 (ones col first).
                    ofull = {}
                    for b in bs:
                        ofull[b] = act.tile([128, 4, PAD + D], F32R, tag="a",
                                            name=f"of{b}")
                        zero_pads(ofull[b])
                    for b in bs:
                        for h in range(H):
                            tcn, prow = h // 2, (h % 2) * DH
                            if O_MODE == "std66":
                                ops = [patt.tile([128, DH + 2], F32, tag="att",
                                                 name=f"o{b}{h}{qc}")
                                       for qc in range(4)]
                                for kc in range(4):
                                    sp = pmm.tile([128, D], F32, tag="mm",
                                                  name=f"s{b}{h}{kc}")
                                    nc.tensor.matmul(
                                        sp[:],
                                        kT[b][prow:prow + DH, tcn,
                                              kc * 128:(kc + 1) * 128],
                                        qT[b][prow:prow + DH, tcn, :],
                                        start=True, stop=True)
                                    ex = expp.tile([128, D], F32R, tag="e",
                                                   name=f"e{b}{h}{kc}")
                                    nc.scalar.activation(ex[:], sp[:], AF.Exp,
                                                         scale=1.0 / np.sqrt(DH))
                                    for qc in range(4):
                                        nc.tensor.matmul(
                                            ops[qc][:],
                                            ex[:, qc * 128:(qc + 1) * 128],
                                            vaug[b][:, kc, h, :],
                                            start=(kc == 0), stop=(kc == 3))
                                for qc in range(4):
                                    rec = stat.tile([128, 1], F32, tag="st",
                                                    name=f"r{b}{h}{qc}")
                                    nc.vector.reciprocal(rec[:],
                                                         ops[qc][:, DH:DH + 1])
                                    nc.vector.tensor_scalar_mul(
                                        ofull[b][:, qc,
                                                 PAD + h * DH:PAD + (h + 1) * DH],
                                        ops[qc][:, 0:DH], rec[:])
                            else:
                                oT = patt.tile([DH + 2, D], F32, tag="att",
                                               name=f"oT{b}{h}")
                                for kc in range(4):
                                    sp = pmm.tile([128, D], F32, tag="mm",
                                                  name=f"s{b}{h}{kc}")
                                    nc.tensor.matmul(
                                        sp[:],
                                        kT[b][prow:prow + DH, tcn,
                                              kc * 128:(kc + 1) * 128],
                                        qT[b][prow:prow + DH, tcn, :],
                                        start=True, stop=True)
                                    ex = expp.tile([128, D], F32R, tag="e",
                                                   name=f"e{b}{h}{kc}")
                                    nc.scalar.activation(ex[:], sp[:], AF.Exp,
                                                         scale=1.0 / np.sqrt(DH))
                                    nc.tensor.matmul(
                                        oT[:], vaug[b][:, kc, h, :], ex[:],
                                        start=(kc == 0), stop=(kc == 3))
                                cs = stat.tile([1, D], F32R, tag="cs",
                                               name=f"cs{b}{h}", bufs=2)
                                nc.scalar.copy(cs[:], oT[DH:DH + 1, :])
                                rcs = stat.tile([1, D], F32R, tag="cs",
                                                name=f"rc{b}{h}", bufs=2)
                                with nc.allow_low_precision(
                                        reason="fp32r softmax denom"):
                                    nc.vector.reciprocal(rcs[:], cs[:])
                                bc = pmm.tile([DH, D], F32, tag="mm",
                                              name=f"bc{b}{h}")
                                nc.tensor.matmul(bc[:], onec[:, 0:DH], rcs[:],
                                                 start=True, stop=True)
                                oTs = expp.tile([DH, D], F32, tag="os",
                                                name=f"oTs{b}{h}", bufs=2)
                                nc.scalar.copy(oTs[:], oT[0:DH, :])
                                oTn = expp.tile([DH, D], F32, tag="on",
                                                name=f"oTn{b}{h}", bufs=2)
                                nc.vector.tensor_mul(oTn[:], oTs[:], bc[:])
                                for qc in range(4):
                                    tp = patt.tile([128, DH], F32, tag="att",
                                                   name=f"ot{b}{h}{qc}")
                                    nc.tensor.transpose(
                                        tp[:], oTn[:, qc * 128:(qc + 1) * 128],
                                        ident[0:DH, 0:DH])
                                    nc.vector.tensor_copy(
                                        ofull[b][:, qc,
                                                 PAD + h * DH:PAD + (h + 1) * DH],
                                        tp[:])

                    # s7: conv_o -> y (fp32)
                    w_t = load_w(wconv, wps["wo"], f"wo{pi}")
                    y = {b: act.tile([128, 4, D], F32, tag="a", name=f"y{b}")
                         for b in bs}

                    def wr_y(b, oc, ps):
                        nc.scalar.activation(y[b][:, oc, :], ps[:], AF.Identity,
                                             bias=bpp[:, 8 + oc:9 + oc])
                    conv_std(bs, w_t, ofull, wr_y)

                    # s8: LN1 (eps/2 absorbs h1 = 2y), transpose, spill to DRAM
                    hn = {b: act.tile([128, 4, D], F32, tag="a", name=f"hn{b}")
                          for b in bs}
                    emit_ln(bs, lnw, stat, y, hn, ln_t["ln1g"], ln_t["ln1b"],
                            EPS / 2, padded_src=False)
                    for b in bs:
                        ht = hpool.tile([128, 4, D], F32R, tag="h",
                                        name=f"hnT{b}")
                        transpose_512(hn[b], ht, f"h{b}")
                        nc.sync.dma_start(
                            hnTd.ap()[b].rearrange("c p dd -> p c dd"), ht[:])

            # ======== FFN phase (all 4 b) ========
            with ExitStack() as fctx:
                w1pool = fctx.enter_context(tc.tile_pool(name="w1pool", bufs=4))
                w2pool = fctx.enter_context(tc.tile_pool(name="w2pool", bufs=16))
                rpool = fctx.enter_context(tc.tile_pool(name="rpool", bufs=2))
                ffh = fctx.enter_context(tc.tile_pool(name="ffh", bufs=2))
                obp = fctx.enter_context(tc.tile_pool(name="obp", bufs=4))
                pff = fctx.enter_context(
                    tc.tile_pool(name="pff", bufs=4, space="PSUM"))

                w1t = []
                for tcn in range(4):
                    t = w1pool.tile([128, DFF], F32R, tag="w1", name=f"w1_{tcn}")
                    nc.sync.dma_start(t[:], w1p.ap()[tcn])
                    w1t.append(t)
                w2t = []
                for fc in range(16):
                    t = w2pool.tile([128, D], F32R, tag="w2", name=f"w2_{fc}")
                    nc.sync.dma_start(t[:], w2p.ap()[fc])
                    w2t.append(t)

                for pi in range(BL // 2):
                    bs = [2 * pi, 2 * pi + 1]
                    hf = {}
                    for b in bs:
                        hf[b] = ffh.tile([128, 4, D], F32R, tag="hf",
                                         name=f"hf{b}")
                        nc.sync.dma_start(
                            hf[b][:], hnTd.ap()[b].rearrange("c p dd -> p c dd"))
                    rl = {b: rpool.tile([128, 16, D], F32R, tag="r",
                                        name=f"rl{b}") for b in bs}
                    for fc in range(16):
                        ps = {b: pff.tile([128, D], F32, tag="f1",
                                          name=f"f{fc}{b}") for b in bs}
                        for tcn in range(4):
                            lhsT = w1t[tcn][:, fc * 128:(fc + 1) * 128]
                            for b in bs:
                                nc.tensor.matmul(ps[b][:], lhsT,
                                                 hf[b][:, tcn, :],
                                                 start=(tcn == 0),
                                                 stop=(tcn == 3))
                        for b in bs:
                            nc.scalar.activation(rl[b][:, fc, :], ps[b][:],
                                                 AF.Relu,
                                                 bias=bpp[:, 12 + fc:13 + fc])
                    for cc in range(4):
                        ps2 = {b: pmm.tile([128, D], F32, tag="mm",
                                           name=f"g{cc}{b}") for b in bs}
                        for b in bs:
                            nc.tensor.matmul(ps2[b][:], onec[:],
                                             brow[:, 1024:1024 + D],
                                             start=True, stop=False)
                        for fc in range(16):
                            rhs = w2t[fc][:]
                            for b in bs:
                                nc.tensor.matmul(
                                    ps2[b][:],
                                    rl[b][:, fc, cc * 128:(cc + 1) * 128], rhs,
                                    start=False, stop=(fc == 15))
                        for b in bs:
                            ob = obp.tile([128, D], F32, tag="ob",
                                          name=f"ob{cc}{b}")
                            nc.scalar.activation(ob[:], ps2[b][:], AF.Copy,
                                                 scale=2.0)
                            nc.sync.dma_start(
                                outp.ap()[b, cc * 128:(cc + 1) * 128, :], ob[:])

    nc.compile()
    return nc


def prep_in_maps(inputs):
    """Full inputs -> list of 8 per-core input dicts (host-side prep)."""
    f = lambda a: np.ascontiguousarray(np.asarray(a, dtype=np.float32))
    x = f(inputs["x"])
    xpad = np.zeros((B, 4, 128, PAD + D), np.float32)
    xpad[:, :, :, PAD:] = x.reshape(B, 4, 128, D)

    shared = {
        "win": _conv_w_host(f(inputs["w_conv_in"])),
        "wq": _conv_w_host(f(inputs["wq"])),
        "wk": _conv_w_host(f(inputs["wk"])),
        "wv": _conv_w_host(f(inputs["wv"])),
        "wo": _conv_w_host(f(inputs["wo"])),
        "w1p": f(inputs["w1"]).reshape(4, 128, DFF),
        "w2p": f(inputs["w2"]).reshape(16, 128, D),
        "browp": np.concatenate(
            [f(inputs["bq"]), f(inputs["bk"]), f(inputs["b2"])])[None, :],
        "onecp": np.ones((1, 128), np.float32),
        "bppp": np.stack(
            [f(inputs["b_conv_in"]).reshape(4, 128)[i] for i in range(4)]
            + [f(inputs["bv"]).reshape(4, 128)[i] for i in range(4)]
            + [f(inputs["bo"]).reshape(4, 128)[i] for i in range(4)]
            + [f(inputs["b1"]).reshape(16, 128)[i] for i in range(16)]
            + [f(inputs["bq"]).reshape(4, 128)[i] for i in range(4)]
            + [f(inputs["bk"]).reshape(4, 128)[i] for i in range(4)],
            axis=1),
        "ln0g": np.tile(f(inputs["ln0_g"]), (128, 1)),
        "ln0b": np.tile(f(inputs["ln0_b"]), (128, 1)),
        "ln1g": np.tile(f(inputs["ln1_g"]), (128, 1)),
        "ln1b": np.tile(f(inputs["ln1_b"]), (128, 1)),
        "onesp": np.concatenate([np.ones((128, 4, 8, 1), np.float32),
                                 np.zeros((128, 4, 8, 1), np.float32)], axis=3),
        "zerosp": np.zeros((128, 4, PAD), np.float32),
    }
    shared = {k: np.ascontiguousarray(v) for k, v in shared.items()}
    return [dict(shared, xp=np.ascontiguousarray(xpad[c * BL:(c + 1) * BL]))
            for c in range(NCORES)]


_NC_CACHE = {}


def get_nc(reps=1):
    if reps not in _NC_CACHE:
        _NC_CACHE[reps] = build_nc(reps)
    return _NC_CACHE[reps]


def kernel(**inputs) -> np.ndarray:
    nc = get_nc()
    in_maps = prep_in_maps(inputs)
    res = run_bass_kernel_spmd(nc, in_maps, list(range(NCORES)))
    return np.concatenate([res.results[c]["outp"] for c in range(NCORES)],
                          axis=0).astype(np.float32)



# revision 2
# speedup vs baseline: 1.2866x; 1.2866x over previous
"""TRN2 Bass kernel for nn_CNN_transformer_hr_xyz_41051297415299.

Reference model (B=32, C=512, D=512, H=8, DFF=2048, K=7), per batch element:
    query_in = causal_conv_in(x)                 # conv over last axis t, mixing C
    xn       = LN0(query_in)                     # over t, (x-m)/(std+eps), ddof=1
    q = conv_q(query_in); k = conv_k(xn); v = conv_v(xn)
    heads split the t axis (8 x 64); attention over the C axis
    o  = softmax(q k^T / 8) v   -> (C, D)
    y  = conv_o(o);  h1 = 2y
    hn = LN1(h1)  ==  LN(y) with eps/2
    out = 2 * (relu(hn @ w1 + b1) @ w2 + b2)

Sharding: data-parallel over batch, 4 per NeuronCore, no collectives.
All matmuls run in bf16 (2x PE rate vs fp32r; rel err ~1e-3 « 2e-2 gate).

Device layout notes (per batch element b):
    std layout  = [channel c (partitions, 4 chunks), t (free)]
    T   layout  = [t (partitions, 4 chunks), channel (free)]
    x, query_in, xn, o_full : std, padded free dim 6+512 (causal left pad)
    qT, kT  : T (conv emitted transposed: lhsT=activation window, rhs=weight)
    v_aug   : [c (part), chunk, head, 66]  (64 v cols + ones col -> softmax
              denominator accumulates in the same matmul as o = p @ v)
"""
import numpy as np
from contextlib import ExitStack

try:
    import concourse.bass as bass
except ImportError:  # pragma: no cover - path fallback for bare containers
    import sys
    for _p in ("/opt/trn_rl_repo", "/root/.axon_site/_ro/trn_rl_repo"):
        if _p not in sys.path:
            sys.path.insert(0, _p)
    import concourse.bass as bass

import ml_dtypes
import concourse.mybir as mybir
import concourse.tile as tile
from concourse import bacc
from concourse.bass_utils import run_bass_kernel_spmd
from concourse.masks import make_identity

B, C, D, H, DFF, KW = 32, 512, 512, 8, 2048, 7
NCORES = 8
BL = B // NCORES          # 4 batch elements per core
DH = D // H               # 64
PAD = KW - 1              # 6
EPS = 1e-6
F32 = mybir.dt.float32
BF16 = mybir.dt.bfloat16
NPBF = ml_dtypes.bfloat16
import os as _os
WBUFS = int(_os.environ.get("K_WBUFS", "5"))
AF = mybir.ActivationFunctionType
ALU = mybir.AluOpType


def _conv_w_host(w):
    """(cout, cin, KW) -> (4, 128, KW*512): [ci][p][k*512+cout]."""
    return np.ascontiguousarray(
        w.transpose(1, 2, 0).reshape(4, 128, KW * C).astype(NPBF))


def build_nc(reps=1):
    nc = bacc.Bacc("TRN2", target_bir_lowering=False, debug=False)

    xp = nc.declare_dram_parameter("xp", [BL, 4, 128, PAD + D], BF16, isOutput=False)
    wps = {n: nc.declare_dram_parameter(n, [4, 128, KW * C], BF16, isOutput=False)
           for n in ("win", "wq", "wk", "wv", "wo")}
    w1p = nc.declare_dram_parameter("w1p", [4, 128, DFF], BF16, isOutput=False)
    w2p = nc.declare_dram_parameter("w2p", [16, 128, D], BF16, isOutput=False)
    browp = nc.declare_dram_parameter("browp", [1, 3 * 512], BF16, isOutput=False)
    onecp = nc.declare_dram_parameter("onecp", [1, 128], BF16, isOutput=False)
    bppp = nc.declare_dram_parameter("bppp", [128, 36], F32, isOutput=False)
    lnp = {n: nc.declare_dram_parameter(n, [128, D], F32, isOutput=False)
           for n in ("ln0g", "ln0b", "ln1g", "ln1b")}
    onesp = nc.declare_dram_parameter("onesp", [128, 4, 8, 2], BF16, isOutput=False)
    zerosp = nc.declare_dram_parameter("zerosp", [128, 4, PAD], BF16, isOutput=False)
    outp = nc.declare_dram_parameter("outp", [BL, C, D], F32, isOutput=True)
    hnTd = nc.dram_tensor("hnTd", [BL, 4, 128, D], BF16)

    with tile.TileContext(nc) as tc, ExitStack() as octx:
        cp = octx.enter_context(tc.tile_pool(name="consts", bufs=1))
        pmm = octx.enter_context(tc.tile_pool(name="pmm", bufs=4, space="PSUM"))

        def ctile(name, shape, dtype, src):
            t = cp.tile(shape, dtype, tag=name, name=name)
            nc.sync.dma_start(t[:], src)
            return t

        brow = ctile("brow", [1, 3 * 512], BF16, browp.ap())
        onec = ctile("onec", [1, 128], BF16, onecp.ap())
        bpp = ctile("bpp", [128, 36], F32, bppp.ap())
        ln_t = {n: ctile(n, [128, D], F32, lnp[n].ap()) for n in lnp}
        ones_t = ctile("ones", [128, 4, 8, 2], BF16, onesp.ap())
        zeros_t = ctile("zeros", [128, 4, PAD], BF16, zerosp.ap())
        ident = cp.tile([128, 128], F32, tag="ident", name="ident")
        make_identity(nc, ident[:])

        def load_w(pool, param, label):
            ts = []
            for ci in range(4):
                t = pool.tile([128, KW * C], BF16, tag="w", name=f"{label}{ci}")
                nc.sync.dma_start(t[:], param.ap()[ci])
                ts.append(t)
            return ts

        def conv_std(bs, wt, src, writer):
            """std conv: out[cout, t] accumulated over (cin chunk, tap);
            weight lhsT reused across the batch pair."""
            for oc in range(4):
                ps = {b: pmm.tile([128, D], F32, tag="mm", name=f"cs{oc}{b}")
                      for b in bs}
                for ci in range(4):
                    for k in range(KW):
                        lhsT = wt[ci][:, k * C + oc * 128: k * C + oc * 128 + 128]
                        for b in bs:
                            nc.tensor.matmul(
                                ps[b][:], lhsT, src[b][:, ci, k:k + D],
                                start=(ci == 0 and k == 0),
                                stop=(ci == 3 and k == KW - 1))
                for b in bs:
                    writer(b, oc, ps[b])

        def conv_T(bs, wt, src, brow_off, dst):
            """transposed conv: out[t, cout]; rank-1 bias matmul first."""
            for tcn in range(4):
                ps = {b: pmm.tile([128, D], F32, tag="mm", name=f"cT{tcn}{b}")
                      for b in bs}
                for b in bs:
                    nc.tensor.matmul(ps[b][:], onec[:],
                                     brow[:, brow_off:brow_off + D],
                                     start=True, stop=False)
                for ci in range(4):
                    for k in range(KW):
                        rhs = wt[ci][:, k * C:(k + 1) * C]
                        for b in bs:
                            lhsT = src[b][:, ci, tcn * 128 + k: tcn * 128 + k + 128]
                            nc.tensor.matmul(ps[b][:], lhsT, rhs, start=False,
                                             stop=(ci == 3 and k == KW - 1))
                for b in bs:
                    nc.vector.tensor_copy(dst[b][:, tcn, :], ps[b][:])

        def transpose_512(src_t, dst_t, label):
            """[c-chunks, t] std tile -> [t-chunks, c] tile via 16 PE transposes."""
            for tcn in range(4):
                for cc in range(4):
                    tp = patt.tile([128, 128], F32, tag="att",
                                   name=f"tp{label}{tcn}{cc}")
                    nc.tensor.transpose(
                        tp[:], src_t[:, cc, tcn * 128:(tcn + 1) * 128], ident[:])
                    nc.vector.tensor_copy(
                        dst_t[:, tcn, cc * 128:(cc + 1) * 128], tp[:])

        def emit_ln(bs, lnw, stat, src, dst, g_t, b_t, eps, padded_src):
            for b in bs:
                for c in range(4):
                    sv = (src[b][:, c, PAD:PAD + D] if padded_src
                          else src[b][:, c, :])
                    sm = stat.tile([128, 1], F32, tag="st", name=f"sm{b}{c}")
                    nc.vector.reduce_sum(sm[:], sv, axis=mybir.AxisListType.X)
                    mn = stat.tile([128, 1], F32, tag="st", name=f"mn{b}{c}")
                    nc.scalar.mul(mn[:], sm[:], 1.0 / D)
                    cent = lnw.tile([128, D], F32, tag="lw", name=f"ce{b}{c}")
                    nc.vector.tensor_scalar(cent[:], sv, mn[:], None,
                                            op0=ALU.subtract)
                    scr = lnw.tile([128, D], F32, tag="lw", name=f"sc{b}{c}")
                    sq = stat.tile([128, 1], F32, tag="st", name=f"sq{b}{c}")
                    nc.scalar.activation(scr[:], cent[:], AF.Square,
                                         accum_out=sq[:])
                    st = stat.tile([128, 1], F32, tag="st", name=f"sd{b}{c}")
                    nc.scalar.activation(st[:], sq[:], AF.Sqrt,
                                         scale=1.0 / (D - 1))
                    dn = stat.tile([128, 1], F32, tag="st", name=f"dn{b}{c}")
                    nc.vector.tensor_scalar_add(dn[:], st[:], eps)
                    iv = stat.tile([128, 1], F32, tag="st", name=f"iv{b}{c}")
                    nc.vector.reciprocal(iv[:], dn[:])
                    tmp = lnw.tile([128, D], F32, tag="lw", name=f"tm{b}{c}")
                    nc.vector.scalar_tensor_tensor(
                        tmp[:], in0=cent[:], scalar=iv[:], in1=g_t[:],
                        op0=ALU.mult, op1=ALU.mult)
                    dv = (dst[b][:, c, PAD:PAD + D] if padded_src
                          else dst[b][:, c, :])
                    nc.vector.tensor_add(dv, tmp[:], b_t[:])

        def zero_pads(t):
            nc.scalar.copy(t[:, :, 0:PAD], zeros_t[:])

        for _rep in range(reps):
            # ======== two passes over batch pairs ========
            with ExitStack() as pctx:
                wconv = pctx.enter_context(tc.tile_pool(name="wconv", bufs=WBUFS))
                act = pctx.enter_context(tc.tile_pool(name="act", bufs=8))
                expp = pctx.enter_context(tc.tile_pool(name="expp", bufs=3))
                lnw = pctx.enter_context(tc.tile_pool(name="lnw", bufs=2))
                stat = pctx.enter_context(tc.tile_pool(name="stat", bufs=16))
                hpool = pctx.enter_context(tc.tile_pool(name="hpool", bufs=2))
                patt = pctx.enter_context(
                    tc.tile_pool(name="patt", bufs=4, space="PSUM"))

                for pi in range(BL // 2):
                    bs = [2 * pi, 2 * pi + 1]
                    # s1: conv_in
                    x_t = {}
                    for b in bs:
                        x_t[b] = act.tile([128, 4, PAD + D], BF16, tag="a",
                                          name=f"x{b}")
                        nc.sync.dma_start(
                            x_t[b][:], xp.ap()[b].rearrange("c p t -> p c t"))
                    w_t = load_w(wconv, wps["win"], f"win{pi}")
                    qin = {}
                    for b in bs:
                        qin[b] = act.tile([128, 4, PAD + D], BF16, tag="a",
                                          name=f"qin{b}")
                        zero_pads(qin[b])

                    def wr_qin(b, oc, ps):
                        nc.scalar.activation(qin[b][:, oc, PAD:PAD + D], ps[:],
                                             AF.Identity, bias=bpp[:, oc:oc + 1])
                    conv_std(bs, w_t, x_t, wr_qin)

                    # s2: LN0
                    xn = {}
                    for b in bs:
                        xn[b] = act.tile([128, 4, PAD + D], BF16, tag="a",
                                         name=f"xn{b}")
                        zero_pads(xn[b])
                    emit_ln(bs, lnw, stat, qin, xn, ln_t["ln0g"], ln_t["ln0b"],
                            EPS, padded_src=True)

                    # s3/s4: conv_q / conv_k -> qT, kT
                    qT = {b: act.tile([128, 4, D], BF16, tag="a",
                                      name=f"qT{b}") for b in bs}
                    kT = {b: act.tile([128, 4, D], BF16, tag="a",
                                      name=f"kT{b}") for b in bs}
                    w_t = load_w(wconv, wps["wq"], f"wq{pi}")
                    conv_T(bs, w_t, qin, 0, qT)
                    w_t = load_w(wconv, wps["wk"], f"wk{pi}")
                    conv_T(bs, w_t, xn, 512, kT)

                    # s5: conv_v -> v_aug; per-head cols: [v0..63, ones, zero]
                    w_t = load_w(wconv, wps["wv"], f"wv{pi}")
                    vaug = {}
                    for b in bs:
                        vaug[b] = act.tile([128, 4, H, DH + 2], BF16, tag="a",
                                           name=f"vaug{b}")
                        nc.scalar.copy(vaug[b][:, :, :, DH:DH + 2],
                                       ones_t[:])

                    def wr_v(b, oc, ps):
                        nc.scalar.activation(
                            vaug[b][:, oc, :, 0:DH],
                            ps[:].rearrange("p (h dd) -> p h dd", h=H),
                            AF.Identity, bias=bpp[:, 4 + oc:5 + oc])
                    conv_std(bs, w_t, xn, wr_v)

                    # s6: attention; scores block = 128 keys x 512 queries,
                    # o accumulated per query chunk (colsum rides along via the
                    # ones column -> softmax denominator).
                    ofull = {}
                    for b in bs:
                        ofull[b] = act.tile([128, 4, PAD + D], BF16, tag="a",
                                            name=f"of{b}")
                        zero_pads(ofull[b])
                    for b in bs:
                        for h in range(H):
                            tcn, prow = h // 2, (h % 2) * DH
                            ops = [patt.tile([128, DH + 2], F32, tag="att",
                                             name=f"o{b}{h}{qc}")
                                   for qc in range(4)]
                            for kc in range(4):
                                sp = pmm.tile([128, D], F32, tag="mm",
                                              name=f"s{b}{h}{kc}")
                                nc.tensor.matmul(
                                    sp[:],
                                    kT[b][prow:prow + DH, tcn,
                                          kc * 128:(kc + 1) * 128],
                                    qT[b][prow:prow + DH, tcn, :],
                                    start=True, stop=True)
                                ex = expp.tile([128, D], BF16, tag="e",
                                               name=f"e{b}{h}{kc}")
                                nc.scalar.activation(ex[:], sp[:], AF.Exp,
                                                     scale=1.0 / np.sqrt(DH))
                                for qc in range(4):
                                    nc.tensor.matmul(
                                        ops[qc][:],
                                        ex[:, qc * 128:(qc + 1) * 128],
                                        vaug[b][:, kc, h, :],
                                        start=(kc == 0), stop=(kc == 3))
                            for qc in range(4):
                                rec = stat.tile([128, 1], F32, tag="st",
                                                name=f"r{b}{h}{qc}")
                                nc.vector.reciprocal(rec[:],
                                                     ops[qc][:, DH:DH + 1])
                                nc.vector.tensor_scalar_mul(
                                    ofull[b][:, qc,
                                             PAD + h * DH:PAD + (h + 1) * DH],
                                    ops[qc][:, 0:DH], rec[:])

                    # s7: conv_o -> y (fp32)
                    w_t = load_w(wconv, wps["wo"], f"wo{pi}")
                    y = {b: act.tile([128, 4, D], F32, tag="a", name=f"y{b}")
                         for b in bs}

                    def wr_y(b, oc, ps):
                        nc.scalar.activation(y[b][:, oc, :], ps[:], AF.Identity,
                                             bias=bpp[:, 8 + oc:9 + oc])
                    conv_std(bs, w_t, ofull, wr_y)

                    # s8: LN1 (eps/2 absorbs h1 = 2y), transpose, spill to DRAM
                    hn = {b: act.tile([128, 4, D], F32, tag="a", name=f"hn{b}")
                          for b in bs}
                    emit_ln(bs, lnw, stat, y, hn, ln_t["ln1g"], ln_t["ln1b"],
                            EPS / 2, padded_src=False)
                    for b in bs:
                        ht = hpool.tile([128, 4, D], BF16, tag="h",
                                        name=f"hnT{b}")
                        transpose_512(hn[b], ht, f"h{b}")
                        nc.sync.dma_start(
                            hnTd.ap()[b].rearrange("c p dd -> p c dd"), ht[:])

            # ======== FFN phase (all 4 b) ========
            with ExitStack() as fctx:
                w1pool = fctx.enter_context(tc.tile_pool(name="w1pool", bufs=4))
                w2pool = fctx.enter_context(tc.tile_pool(name="w2pool", bufs=16))
                rpool = fctx.enter_context(tc.tile_pool(name="rpool", bufs=2))
                ffh = fctx.enter_context(tc.tile_pool(name="ffh", bufs=2))
                obp = fctx.enter_context(tc.tile_pool(name="obp", bufs=4))
                pff = fctx.enter_context(
                    tc.tile_pool(name="pff", bufs=4, space="PSUM"))

                w1t = []
                for tcn in range(4):
                    t = w1pool.tile([128, DFF], BF16, tag="w1", name=f"w1_{tcn}")
                    nc.sync.dma_start(t[:], w1p.ap()[tcn])
                    w1t.append(t)
                w2t = []
                for fc in range(16):
                    t = w2pool.tile([128, D], BF16, tag="w2", name=f"w2_{fc}")
                    nc.sync.dma_start(t[:], w2p.ap()[fc])
                    w2t.append(t)

                for pi in range(BL // 2):
                    bs = [2 * pi, 2 * pi + 1]
                    hf = {}
                    for b in bs:
                        hf[b] = ffh.tile([128, 4, D], BF16, tag="hf",
                                         name=f"hf{b}")
                        nc.sync.dma_start(
                            hf[b][:], hnTd.ap()[b].rearrange("c p dd -> p c dd"))
                    rl = {b: rpool.tile([128, 16, D], BF16, tag="r",
                                        name=f"rl{b}") for b in bs}
                    for fc in range(16):
                        ps = {b: pff.tile([128, D], F32, tag="f1",
                                          name=f"f{fc}{b}") for b in bs}
                        for tcn in range(4):
                            lhsT = w1t[tcn][:, fc * 128:(fc + 1) * 128]
                            for b in bs:
                                nc.tensor.matmul(ps[b][:], lhsT,
                                                 hf[b][:, tcn, :],
                                                 start=(tcn == 0),
                                                 stop=(tcn == 3))
                        for b in bs:
                            nc.scalar.activation(rl[b][:, fc, :], ps[b][:],
                                                 AF.Relu,
                                                 bias=bpp[:, 12 + fc:13 + fc])
                    for cc in range(4):
                        ps2 = {b: pmm.tile([128, D], F32, tag="mm",
                                           name=f"g{cc}{b}") for b in bs}
                        for b in bs:
                            nc.tensor.matmul(ps2[b][:], onec[:],
                                             brow[:, 1024:1024 + D],
                                             start=True, stop=False)
                        for fc in range(16):
                            rhs = w2t[fc][:]
                            for b in bs:
                                nc.tensor.matmul(
                                    ps2[b][:],
                                    rl[b][:, fc, cc * 128:(cc + 1) * 128], rhs,
                                    start=False, stop=(fc == 15))
                        for b in bs:
                            ob = obp.tile([128, D], F32, tag="ob",
                                          name=f"ob{cc}{b}")
                            nc.scalar.activation(ob[:], ps2[b][:], AF.Copy,
                                                 scale=2.0)
                            nc.sync.dma_start(
                                outp.ap()[b, cc * 128:(cc + 1) * 128, :], ob[:])

    nc.compile()
    return nc


def prep_in_maps(inputs):
    """Full inputs -> list of 8 per-core input dicts (host-side prep)."""
    f = lambda a: np.ascontiguousarray(np.asarray(a, dtype=np.float32))
    x = f(inputs["x"])
    xpad = np.zeros((B, 4, 128, PAD + D), NPBF)
    xpad[:, :, :, PAD:] = x.reshape(B, 4, 128, D).astype(NPBF)

    shared = {
        "win": _conv_w_host(f(inputs["w_conv_in"])),
        "wq": _conv_w_host(f(inputs["wq"])),
        "wk": _conv_w_host(f(inputs["wk"])),
        "wv": _conv_w_host(f(inputs["wv"])),
        "wo": _conv_w_host(f(inputs["wo"])),
        "w1p": f(inputs["w1"]).reshape(4, 128, DFF).astype(NPBF),
        "w2p": f(inputs["w2"]).reshape(16, 128, D).astype(NPBF),
        "browp": np.concatenate(
            [f(inputs["bq"]), f(inputs["bk"]), f(inputs["b2"])])[None, :]
            .astype(NPBF),
        "onecp": np.ones((1, 128), NPBF),
        "bppp": np.stack(
            [f(inputs["b_conv_in"]).reshape(4, 128)[i] for i in range(4)]
            + [f(inputs["bv"]).reshape(4, 128)[i] for i in range(4)]
            + [f(inputs["bo"]).reshape(4, 128)[i] for i in range(4)]
            + [f(inputs["b1"]).reshape(16, 128)[i] for i in range(16)]
            + [f(inputs["bq"]).reshape(4, 128)[i] for i in range(4)]
            + [f(inputs["bk"]).reshape(4, 128)[i] for i in range(4)],
            axis=1),
        "ln0g": np.tile(f(inputs["ln0_g"]), (128, 1)),
        "ln0b": np.tile(f(inputs["ln0_b"]), (128, 1)),
        "ln1g": np.tile(f(inputs["ln1_g"]), (128, 1)),
        "ln1b": np.tile(f(inputs["ln1_b"]), (128, 1)),
        "onesp": np.concatenate([np.ones((128, 4, 8, 1), NPBF),
                                 np.zeros((128, 4, 8, 1), NPBF)], axis=3),
        "zerosp": np.zeros((128, 4, PAD), NPBF),
    }
    shared = {k: np.ascontiguousarray(v) for k, v in shared.items()}
    return [dict(shared, xp=np.ascontiguousarray(xpad[c * BL:(c + 1) * BL]))
            for c in range(NCORES)]


_NC_CACHE = {}


def get_nc(reps=1):
    if reps not in _NC_CACHE:
        _NC_CACHE[reps] = build_nc(reps)
    return _NC_CACHE[reps]


def kernel(**inputs) -> np.ndarray:
    nc = get_nc()
    in_maps = prep_in_maps(inputs)
    res = run_bass_kernel_spmd(nc, in_maps, list(range(NCORES)))
    return np.concatenate([res.results[c]["outp"] for c in range(NCORES)],
                          axis=0).astype(np.float32)


# revision 3
# speedup vs baseline: 2.3405x; 1.8191x over previous
"""TRN2 Bass kernel for nn_CNN_transformer_hr_xyz_41051297415299.

Reference model (B=32, C=512, D=512, H=8, DFF=2048, K=7), per batch element:
    query_in = causal_conv_in(x)                 # conv over last axis t, mixing C
    xn       = LN0(query_in)                     # over t, (x-m)/(std+eps), ddof=1
    q = conv_q(query_in); k = conv_k(xn); v = conv_v(xn)
    heads split the t axis (8 x 64); attention over the C axis
    o  = softmax(q k^T / 8) v   -> (C, D)
    y  = conv_o(o);  h1 = 2y
    hn = LN1(h1)  ==  LN(y) with eps/2
    out = 2 * (relu(hn @ w1 + b1) @ w2 + b2)

Sharding: data-parallel over batch, 4 per NeuronCore, no collectives.
All matmuls in bf16 (rel err ~6e-3 « 2e-2 gate). All inputs packed into
two flat DRAM blobs (bf16 + f32) because the per-call staging cost scales
with param count (~32us each) and bytes (~13us/MB); output is bf16 and
upcast on host.

Device layout notes (per batch element b):
    std layout  = [channel c (partitions, 4 chunks), t (free)]
    T   layout  = [t (partitions, 4 chunks), channel (free)]
    x, query_in, xn, o_full : std, padded free dim 6+512 (causal left pad)
    qT, kT  : T (conv emitted transposed: lhsT=activation window, rhs=weight)
    v_aug   : [c (part), chunk, head, 66]  (64 v cols + ones col -> softmax
              denominator accumulates in the same matmul as o = p @ v)
"""
import numpy as np
from contextlib import ExitStack

try:
    import concourse.bass as bass
except ImportError:  # pragma: no cover - path fallback for bare containers
    import sys
    for _p in ("/opt/trn_rl_repo", "/root/.axon_site/_ro/trn_rl_repo"):
        if _p not in sys.path:
            sys.path.insert(0, _p)
    import concourse.bass as bass

import ml_dtypes
import concourse.mybir as mybir
import concourse.tile as tile
from concourse import bacc
from concourse.bass_utils import run_bass_kernel_spmd
from concourse.masks import make_identity

B, C, D, H, DFF, KW = 32, 512, 512, 8, 2048, 7
NCORES = 8
BL = B // NCORES          # 4 batch elements per core
DH = D // H               # 64
PAD = KW - 1              # 6
EPS = 1e-6
F32 = mybir.dt.float32
BF16 = mybir.dt.bfloat16
NPBF = ml_dtypes.bfloat16
AF = mybir.ActivationFunctionType
ALU = mybir.AluOpType

# ---- packed wblob (bf16) layout: name -> (offset_elems, rows, cols) ----
_WREG = {}
_WOFF = 0


def _wreg(name, rows, cols):
    global _WOFF
    _WREG[name] = (_WOFF, rows, cols)
    _WOFF += rows * cols


for _n in ("win", "wq", "wk", "wv", "wo"):
    for _ci in range(4):
        _wreg(f"{_n}{_ci}", 128, KW * C)
for _ci in range(4):
    _wreg(f"w1_{_ci}", 128, DFF)
for _fc in range(16):
    _wreg(f"w2_{_fc}", 128, D)
for _b in range(BL):
    _wreg(f"x{_b}", 128, 4 * (PAD + D))
_wreg("brow", 1, 3 * 512)
WBLOB = _WOFF

# ---- packed fblob (f32) layout ----
_FREG = {}
_FOFF = 0


def _freg(name, rows, cols):
    global _FOFF
    _FREG[name] = (_FOFF, rows, cols)
    _FOFF += rows * cols


_freg("bpp", 128, 36)
for _n in ("ln0g", "ln0b", "ln1g", "ln1b"):
    _freg(_n, 128, D)
FBLOB = _FOFF


def _conv_w_host(w):
    """(cout, cin, KW) -> (4, 128, KW*512): [ci][p][k*512+cout]."""
    return np.ascontiguousarray(
        w.transpose(1, 2, 0).reshape(4, 128, KW * C).astype(NPBF))


def build_nc(reps=1):
    nc = bacc.Bacc("TRN2", target_bir_lowering=False, debug=False)

    wblob = nc.declare_dram_parameter("wblob", [WBLOB], BF16, isOutput=False)
    fblob = nc.declare_dram_parameter("fblob", [FBLOB], F32, isOutput=False)
    outp = nc.declare_dram_parameter("outp", [BL, C, D], BF16, isOutput=True)
    hnTd = nc.dram_tensor("hnTd", [BL, 4, 128, D], BF16)

    def wsrc(name):
        off, r, c = _WREG[name]
        return wblob.ap()[off:off + r * c].rearrange("(p t) -> p t", p=r)

    def fsrc(name):
        off, r, c = _FREG[name]
        return fblob.ap()[off:off + r * c].rearrange("(p t) -> p t", p=r)

    with tile.TileContext(nc) as tc, ExitStack() as octx:
        cp = octx.enter_context(tc.tile_pool(name="consts", bufs=1))
        pmm = octx.enter_context(tc.tile_pool(name="pmm", bufs=4, space="PSUM"))

        def ctile(name, shape, dtype, src):
            t = cp.tile(shape, dtype, tag=name, name=name)
            nc.sync.dma_start(t[:], src)
            return t

        brow = ctile("brow", [1, 3 * 512], BF16, wsrc("brow"))
        bpp = ctile("bpp", [128, 36], F32, fsrc("bpp"))
        ln_t = {n: ctile(n, [128, D], F32, fsrc(n))
                for n in ("ln0g", "ln0b", "ln1g", "ln1b")}
        onec = cp.tile([1, 128], BF16, tag="onec", name="onec")
        nc.gpsimd.memset(onec[:], 1.0)
        ident = cp.tile([128, 128], F32, tag="ident", name="ident")
        make_identity(nc, ident[:])

        def load_w(pool, wname, label):
            ts = []
            for ci in range(4):
                t = pool.tile([128, KW * C], BF16, tag="w", name=f"{label}{ci}")
                nc.sync.dma_start(t[:], wsrc(f"{wname}{ci}"))
                ts.append(t)
            return ts

        def conv_std(bs, wt, src, writer):
            """std conv: out[cout, t] accumulated over (cin chunk, tap);
            weight lhsT reused across the batch pair."""
            for oc in range(4):
                ps = {b: pmm.tile([128, D], F32, tag="mm", name=f"cs{oc}{b}")
                      for b in bs}
                for ci in range(4):
                    for k in range(KW):
                        lhsT = wt[ci][:, k * C + oc * 128: k * C + oc * 128 + 128]
                        for b in bs:
                            nc.tensor.matmul(
                                ps[b][:], lhsT, src[b][:, ci, k:k + D],
                                start=(ci == 0 and k == 0),
                                stop=(ci == 3 and k == KW - 1))
                for b in bs:
                    writer(b, oc, ps[b])

        def conv_T(bs, wt, src, brow_off, dst):
            """transposed conv: out[t, cout]; rank-1 bias matmul first."""
            for tcn in range(4):
                ps = {b: pmm.tile([128, D], F32, tag="mm", name=f"cT{tcn}{b}")
                      for b in bs}
                for b in bs:
                    nc.tensor.matmul(ps[b][:], onec[:],
                                     brow[:, brow_off:brow_off + D],
                                     start=True, stop=False)
                for ci in range(4):
                    for k in range(KW):
                        rhs = wt[ci][:, k * C:(k + 1) * C]
                        for b in bs:
                            lhsT = src[b][:, ci, tcn * 128 + k: tcn * 128 + k + 128]
                            nc.tensor.matmul(ps[b][:], lhsT, rhs, start=False,
                                             stop=(ci == 3 and k == KW - 1))
                for b in bs:
                    nc.vector.tensor_copy(dst[b][:, tcn, :], ps[b][:])

        def transpose_512(src_t, dst_t, label):
            """[c-chunks, t] std tile -> [t-chunks, c] tile via 16 PE transposes."""
            for tcn in range(4):
                for cc in range(4):
                    tp = patt.tile([128, 128], F32, tag="att",
                                   name=f"tp{label}{tcn}{cc}")
                    nc.tensor.transpose(
                        tp[:], src_t[:, cc, tcn * 128:(tcn + 1) * 128], ident[:])
                    nc.vector.tensor_copy(
                        dst_t[:, tcn, cc * 128:(cc + 1) * 128], tp[:])

        def emit_ln(bs, lnw, stat, src, dst, g_t, b_t, eps, padded_src):
            for b in bs:
                for c in range(4):
                    sv = (src[b][:, c, PAD:PAD + D] if padded_src
                          else src[b][:, c, :])
                    sm = stat.tile([128, 1], F32, tag="st", name=f"sm{b}{c}")
                    nc.vector.reduce_sum(sm[:], sv, axis=mybir.AxisListType.X)
                    mn = stat.tile([128, 1], F32, tag="st", name=f"mn{b}{c}")
                    nc.scalar.mul(mn[:], sm[:], 1.0 / D)
                    cent = lnw.tile([128, D], F32, tag="lw", name=f"ce{b}{c}")
                    nc.vector.tensor_scalar(cent[:], sv, mn[:], None,
                                            op0=ALU.subtract)
                    scr = lnw.tile([128, D], F32, tag="lw", name=f"sc{b}{c}")
                    sq = stat.tile([128, 1], F32, tag="st", name=f"sq{b}{c}")
                    nc.scalar.activation(scr[:], cent[:], AF.Square,
                                         accum_out=sq[:])
                    st = stat.tile([128, 1], F32, tag="st", name=f"sd{b}{c}")
                    nc.scalar.activation(st[:], sq[:], AF.Sqrt,
                                         scale=1.0 / (D - 1))
                    dn = stat.tile([128, 1], F32, tag="st", name=f"dn{b}{c}")
                    nc.vector.tensor_scalar_add(dn[:], st[:], eps)
                    iv = stat.tile([128, 1], F32, tag="st", name=f"iv{b}{c}")
                    nc.vector.reciprocal(iv[:], dn[:])
                    tmp = lnw.tile([128, D], F32, tag="lw", name=f"tm{b}{c}")
                    nc.vector.scalar_tensor_tensor(
                        tmp[:], in0=cent[:], scalar=iv[:], in1=g_t[:],
                        op0=ALU.mult, op1=ALU.mult)
                    dv = (dst[b][:, c, PAD:PAD + D] if padded_src
                          else dst[b][:, c, :])
                    nc.vector.tensor_add(dv, tmp[:], b_t[:])

        def zero_pads(t):
            nc.gpsimd.memset(t[:, :, 0:PAD], 0.0)

        for _rep in range(reps):
            # ======== two passes over batch pairs ========
            with ExitStack() as pctx:
                wconv = pctx.enter_context(tc.tile_pool(name="wconv", bufs=5))
                act = pctx.enter_context(tc.tile_pool(name="act", bufs=8))
                expp = pctx.enter_context(tc.tile_pool(name="expp", bufs=3))
                lnw = pctx.enter_context(tc.tile_pool(name="lnw", bufs=2))
                stat = pctx.enter_context(tc.tile_pool(name="stat", bufs=16))
                hpool = pctx.enter_context(tc.tile_pool(name="hpool", bufs=2))
                patt = pctx.enter_context(
                    tc.tile_pool(name="patt", bufs=4, space="PSUM"))

                for pi in range(BL // 2):
                    bs = [2 * pi, 2 * pi + 1]
                    # s1: conv_in
                    x_t = {}
                    for b in bs:
                        x_t[b] = act.tile([128, 4, PAD + D], BF16, tag="a",
                                          name=f"x{b}")
                        nc.sync.dma_start(
                            x_t[b][:],
                            wsrc(f"x{b}").rearrange("p (c t) -> p c t", c=4))
                    w_t = load_w(wconv, "win", f"win{pi}")
                    qin = {}
                    for b in bs:
                        qin[b] = act.tile([128, 4, PAD + D], BF16, tag="a",
                                          name=f"qin{b}")
                        zero_pads(qin[b])

                    def wr_qin(b, oc, ps):
                        nc.scalar.activation(qin[b][:, oc, PAD:PAD + D], ps[:],
                                             AF.Identity, bias=bpp[:, oc:oc + 1])
                    conv_std(bs, w_t, x_t, wr_qin)

                    # s2: LN0
                    xn = {}
                    for b in bs:
                        xn[b] = act.tile([128, 4, PAD + D], BF16, tag="a",
                                         name=f"xn{b}")
                        zero_pads(xn[b])
                    emit_ln(bs, lnw, stat, qin, xn, ln_t["ln0g"], ln_t["ln0b"],
                            EPS, padded_src=True)

                    # s3/s4: conv_q / conv_k -> qT, kT
                    qT = {b: act.tile([128, 4, D], BF16, tag="a",
                                      name=f"qT{b}") for b in bs}
                    kT = {b: act.tile([128, 4, D], BF16, tag="a",
                                      name=f"kT{b}") for b in bs}
                    w_t = load_w(wconv, "wq", f"wq{pi}")
                    conv_T(bs, w_t, qin, 0, qT)
                    w_t = load_w(wconv, "wk", f"wk{pi}")
                    conv_T(bs, w_t, xn, 512, kT)

                    # s5: conv_v -> v_aug; per-head cols: [v0..63, one, zero]
                    w_t = load_w(wconv, "wv", f"wv{pi}")
                    vaug = {}
                    for b in bs:
                        vaug[b] = act.tile([128, 4, H, DH + 2], BF16, tag="a",
                                           name=f"vaug{b}")
                        nc.gpsimd.memset(vaug[b][:, :, :, DH:DH + 1], 1.0)
                        nc.gpsimd.memset(vaug[b][:, :, :, DH + 1:DH + 2], 0.0)

                    def wr_v(b, oc, ps):
                        nc.scalar.activation(
                            vaug[b][:, oc, :, 0:DH],
                            ps[:].rearrange("p (h dd) -> p h dd", h=H),
                            AF.Identity, bias=bpp[:, 4 + oc:5 + oc])
                    conv_std(bs, w_t, xn, wr_v)

                    # s6: attention; scores block = 128 keys x 512 queries,
                    # o accumulated per query chunk (colsum rides along via the
                    # ones column -> softmax denominator).
                    ofull = {}
                    for b in bs:
                        ofull[b] = act.tile([128, 4, PAD + D], BF16, tag="a",
                                            name=f"of{b}")
                        zero_pads(ofull[b])
                    for b in bs:
                        for h in range(H):
                            tcn, prow = h // 2, (h % 2) * DH
                            ops = [patt.tile([128, DH + 2], F32, tag="att",
                                             name=f"o{b}{h}{qc}")
                                   for qc in range(4)]
                            for kc in range(4):
                                sp = pmm.tile([128, D], F32, tag="mm",
                                              name=f"s{b}{h}{kc}")
                                nc.tensor.matmul(
                                    sp[:],
                                    kT[b][prow:prow + DH, tcn,
                                          kc * 128:(kc + 1) * 128],
                                    qT[b][prow:prow + DH, tcn, :],
                                    start=True, stop=True)
                                ex = expp.tile([128, D], BF16, tag="e",
                                               name=f"e{b}{h}{kc}")
                                nc.scalar.activation(ex[:], sp[:], AF.Exp,
                                                     scale=1.0 / np.sqrt(DH))
                                for qc in range(4):
                                    nc.tensor.matmul(
                                        ops[qc][:],
                                        ex[:, qc * 128:(qc + 1) * 128],
                                        vaug[b][:, kc, h, :],
                                        start=(kc == 0), stop=(kc == 3))
                            for qc in range(4):
                                rec = stat.tile([128, 1], F32, tag="st",
                                                name=f"r{b}{h}{qc}")
                                nc.vector.reciprocal(rec[:],
                                                     ops[qc][:, DH:DH + 1])
                                nc.vector.tensor_scalar_mul(
                                    ofull[b][:, qc,
                                             PAD + h * DH:PAD + (h + 1) * DH],
                                    ops[qc][:, 0:DH], rec[:])

                    # s7: conv_o -> y (fp32)
                    w_t = load_w(wconv, "wo", f"wo{pi}")
                    y = {b: act.tile([128, 4, D], F32, tag="a", name=f"y{b}")
                         for b in bs}

                    def wr_y(b, oc, ps):
                        nc.scalar.activation(y[b][:, oc, :], ps[:], AF.Identity,
                                             bias=bpp[:, 8 + oc:9 + oc])
                    conv_std(bs, w_t, ofull, wr_y)

                    # s8: LN1 (eps/2 absorbs h1 = 2y), transpose, spill to DRAM
                    hn = {b: act.tile([128, 4, D], F32, tag="a", name=f"hn{b}")
                          for b in bs}
                    emit_ln(bs, lnw, stat, y, hn, ln_t["ln1g"], ln_t["ln1b"],
                            EPS / 2, padded_src=False)
                    for b in bs:
                        ht = hpool.tile([128, 4, D], BF16, tag="h",
                                        name=f"hnT{b}")
                        transpose_512(hn[b], ht, f"h{b}")
                        nc.sync.dma_start(
                            hnTd.ap()[b].rearrange("c p dd -> p c dd"), ht[:])

            # ======== FFN phase (all 4 b) ========
            with ExitStack() as fctx:
                w1pool = fctx.enter_context(tc.tile_pool(name="w1pool", bufs=4))
                w2pool = fctx.enter_context(tc.tile_pool(name="w2pool", bufs=16))
                rpool = fctx.enter_context(tc.tile_pool(name="rpool", bufs=2))
                ffh = fctx.enter_context(tc.tile_pool(name="ffh", bufs=2))
                obp = fctx.enter_context(tc.tile_pool(name="obp", bufs=4))
                pff = fctx.enter_context(
                    tc.tile_pool(name="pff", bufs=4, space="PSUM"))

                w1t = []
                for tcn in range(4):
                    t = w1pool.tile([128, DFF], BF16, tag="w1", name=f"w1_{tcn}")
                    nc.sync.dma_start(t[:], wsrc(f"w1_{tcn}"))
                    w1t.append(t)
                w2t = []
                for fc in range(16):
                    t = w2pool.tile([128, D], BF16, tag="w2", name=f"w2_{fc}")
                    nc.sync.dma_start(t[:], wsrc(f"w2_{fc}"))
                    w2t.append(t)

                for pi in range(BL // 2):
                    bs = [2 * pi, 2 * pi + 1]
                    hf = {}
                    for b in bs:
                        hf[b] = ffh.tile([128, 4, D], BF16, tag="hf",
                                         name=f"hf{b}")
                        nc.sync.dma_start(
                            hf[b][:], hnTd.ap()[b].rearrange("c p dd -> p c dd"))
                    rl = {b: rpool.tile([128, 16, D], BF16, tag="r",
                                        name=f"rl{b}") for b in bs}
                    for fc in range(16):
                        ps = {b: pff.tile([128, D], F32, tag="f1",
                                          name=f"f{fc}{b}") for b in bs}
                        for tcn in range(4):
                            lhsT = w1t[tcn][:, fc * 128:(fc + 1) * 128]
                            for b in bs:
                                nc.tensor.matmul(ps[b][:], lhsT,
                                                 hf[b][:, tcn, :],
                                                 start=(tcn == 0),
                                                 stop=(tcn == 3))
                        for b in bs:
                            nc.scalar.activation(rl[b][:, fc, :], ps[b][:],
                                                 AF.Relu,
                                                 bias=bpp[:, 12 + fc:13 + fc])
                    for cc in range(4):
                        ps2 = {b: pmm.tile([128, D], F32, tag="mm",
                                           name=f"g{cc}{b}") for b in bs}
                        for b in bs:
                            nc.tensor.matmul(ps2[b][:], onec[:],
                                             brow[:, 1024:1024 + D],
                                             start=True, stop=False)
                        for fc in range(16):
                            rhs = w2t[fc][:]
                            for b in bs:
                                nc.tensor.matmul(
                                    ps2[b][:],
                                    rl[b][:, fc, cc * 128:(cc + 1) * 128], rhs,
                                    start=False, stop=(fc == 15))
                        for b in bs:
                            ob = obp.tile([128, D], BF16, tag="ob",
                                          name=f"ob{cc}{b}")
                            nc.scalar.activation(ob[:], ps2[b][:], AF.Copy,
                                                 scale=2.0)
                            nc.sync.dma_start(
                                outp.ap()[b, cc * 128:(cc + 1) * 128, :], ob[:])

    nc.compile()
    return nc


def prep_in_maps(inputs):
    """Full inputs -> list of 8 per-core input dicts (host-side prep)."""
    f = lambda a: np.ascontiguousarray(np.asarray(a, dtype=np.float32))
    x = f(inputs["x"])
    # per-core x, pre-transposed to [128, 4ci, PAD+D] and causal-padded
    xpad = np.zeros((B, 128, 4, PAD + D), NPBF)
    xpad[:, :, :, PAD:] = x.reshape(B, 4, 128, D).transpose(0, 2, 1, 3) \
        .astype(NPBF)

    wparts = {}
    for n, key in (("win", "w_conv_in"), ("wq", "wq"), ("wk", "wk"),
                   ("wv", "wv"), ("wo", "wo")):
        cw = _conv_w_host(f(inputs[key]))
        for ci in range(4):
            wparts[f"{n}{ci}"] = cw[ci]
    w1 = f(inputs["w1"]).reshape(4, 128, DFF).astype(NPBF)
    for ci in range(4):
        wparts[f"w1_{ci}"] = w1[ci]
    w2 = f(inputs["w2"]).reshape(16, 128, D).astype(NPBF)
    for fc in range(16):
        wparts[f"w2_{fc}"] = w2[fc]
    wparts["brow"] = np.concatenate(
        [f(inputs["bq"]), f(inputs["bk"]), f(inputs["b2"])])[None, :] \
        .astype(NPBF)

    fb = np.empty(FBLOB, np.float32)
    fparts = {
        "bpp": np.stack(
            [f(inputs["b_conv_in"]).reshape(4, 128)[i] for i in range(4)]
            + [f(inputs["bv"]).reshape(4, 128)[i] for i in range(4)]
            + [f(inputs["bo"]).reshape(4, 128)[i] for i in range(4)]
            + [f(inputs["b1"]).reshape(16, 128)[i] for i in range(16)]
            + [f(inputs["bq"]).reshape(4, 128)[i] for i in range(4)]
            + [f(inputs["bk"]).reshape(4, 128)[i] for i in range(4)],
            axis=1),
        "ln0g": np.tile(f(inputs["ln0_g"]), (128, 1)),
        "ln0b": np.tile(f(inputs["ln0_b"]), (128, 1)),
        "ln1g": np.tile(f(inputs["ln1_g"]), (128, 1)),
        "ln1b": np.tile(f(inputs["ln1_b"]), (128, 1)),
    }
    for n, (off, r, c) in _FREG.items():
        fb[off:off + r * c] = fparts[n].reshape(-1)

    maps = []
    for core in range(NCORES):
        wb = np.zeros(WBLOB, NPBF)
        for n, (off, r, c) in _WREG.items():
            if n.startswith("x"):
                b = int(n[1:])
                wb[off:off + r * c] = xpad[core * BL + b].reshape(-1)
            elif n == "brow":
                wb[off:off + 3 * 512] = wparts["brow"].reshape(-1)
            else:
                wb[off:off + r * c] = wparts[n].reshape(-1)
        maps.append({"wblob": wb, "fblob": fb})
    return maps


_NC_CACHE = {}


def get_nc(reps=1):
    if reps not in _NC_CACHE:
        _NC_CACHE[reps] = build_nc(reps)
    return _NC_CACHE[reps]


def kernel(**inputs) -> np.ndarray:
    nc = get_nc()
    in_maps = prep_in_maps(inputs)
    res = run_bass_kernel_spmd(nc, in_maps, list(range(NCORES)))
    return np.concatenate([res.results[c]["outp"] for c in range(NCORES)],
                          axis=0).astype(np.float32)
